# revision 1
# baseline (speedup 1.0000x reference)
"""Multi-head attention (B=2, S=2048, H=16, DH=64, D=1024) on 8 TRN2 NeuronCores.

Sharding: batch x head-group. Core c handles batch b = c//4, head group
hg = c%4 (4 heads = 256 hidden columns). Each core computes its head group's
attention and a partial (row-sliced) output projection; the host sums the 4
partials per batch and adds the bias terms.

Device-side dataflow (per core), v2:
  - All projection operands (hsT, Wq/Wk/Wv/Wo) arrive bf16 (halves input DMA
    vs fp32r; PE rate identical). PSUM accumulation fp32 throughout.
  - qT/kT [256, S] bf16 via Wq/Wk-contract matmuls; bias added by the ScalarE
    Identity activation that evacuates PSUM.
  - v = hs @ Wv stored as fp8(e4m3) in a DoubleRow-paired layout
    v_sb[128, sp, j, h, 80]: sk-chunk pair sp = skc//2, j = skc%2, with a
    ones column at col 64 per (j, h) slot so the PV matmul also produces the
    softmax denominator l as ctx row 64.
  - Per head pair and sk chunk: two K=64 scoresT matmuls (bf16) -> one
    [128, 1024] psum tile; exp is computed chunk-by-chunk into a paired fp8
    tile expT[128, j, head, 512]: most chunks on ScalarE (Exp activation,
    fp8 out), a subset on the DVE as a one-instruction Schraudolph
    exp-to-e4m3: int8(x*(SCALE*8/ln2) + 56.x) bitcast to fp8 (sawtooth err
    ~2-3%, cancelled to first order by the shared denominator).
  - PV: fp8 DoubleRow matmuls, K_eff=256 (chunk pair packed along the free
    dim of both operands): lhsT = v_sb[:, sp, :, h, 0:65], rhs =
    expT[:, :, h, :] -> ctx [65, 512] accumulated over 8 pairs.
  - Epilogue per head: l -> 1/l (DVE reciprocal), broadcast across 64
    partitions via a K=1 matmul, ctxT(bf16) = ctx_unnorm * bcast(1/l).
  - outT_partial [D, S] fp32 = Wo-contract over ctxT (bf16).
Software-pipelined identically to v1 (deferred qT/kT projections, epilogues
and out-projections dripped into later attention units' skc loops).
Host: out[b] = sum_hg(outT_partial).T + (bo + bv @ Wo).
"""

import numpy as np

H = 16
DH = 64
D = 1024
B = 2
S = 2048
HG = 4            # heads per core
DG = HG * DH      # 256 hidden cols per core
SCALE = DH ** -0.5
N_CORES = 8

# skc indices whose exp runs on the DVE (Schraudolph); rest on ScalarE.
# 7/16 on DVE: InstReciprocal measures ~3-5x the 1-pass cost (HW microbench),
# so the epilogue reciprocals pin ~35us on the DVE; balancing pushes the bc
# copies + evacuations to ScalarE and 2 extra exp chunks here.
DVE_EXP_SKC = {2, 4, 6, 8, 10, 12, 14}
# Schraudolph-to-e4m3 constants: code = x*SCALE*(2^3/ln2) + (7*2^3 - c)
SCH_C1 = SCALE * 8.0 / float(np.log(2.0))
SCH_C2 = 56.0 - 0.40  # sawtooth centering; argmin of the full-pipeline numpy
                      # model (flat basin 0.2-0.5, cliff above 0.7)

_cached_nc = None


def _build_nc(reps=1):
    import concourse.bass as bass  # noqa: F401
    from concourse import bacc
    import concourse.mybir as mybir
    import concourse.tile as tile

    F32 = mybir.dt.float32
    F32R = mybir.dt.float32r
    BF16 = mybir.dt.bfloat16
    FP8 = mybir.dt.float8e4
    I8 = mybir.dt.int8
    AFT = mybir.ActivationFunctionType
    ALU = mybir.AluOpType
    DR = mybir.MatmulPerfMode.DoubleRow

    nc = bacc.Bacc("TRN2", target_bir_lowering=False)

    hsT = nc.dram_tensor("hsT", [D, S], BF16, kind="ExternalInput")
    wq = nc.dram_tensor("wq", [D, DG], BF16, kind="ExternalInput")
    wk = nc.dram_tensor("wk", [D, DG], BF16, kind="ExternalInput")
    wv = nc.dram_tensor("wv", [D, DG], BF16, kind="ExternalInput")
    wo = nc.dram_tensor("wo", [DG, D], BF16, kind="ExternalInput")
    bq = nc.dram_tensor("bq", [2, 128], F32, kind="ExternalInput")
    bk = nc.dram_tensor("bk", [2, 128], F32, kind="ExternalInput")
    outT = nc.dram_tensor("outT", [D, S], BF16, kind="ExternalOutput")

    KC = D // 128     # 8 contraction chunks for projections
    SQC = S // 512    # 4 sq chunks of 512
    SKC = S // 128    # 16 sk chunks of 128
    SPC = SKC // 2    # 8 sk chunk-pairs (DoubleRow)

    with tile.TileContext(nc) as tc:
        with tc.tile_pool(name="big", bufs=1) as big, \
             tc.tile_pool(name="expp", bufs=4) as expp, \
             tc.tile_pool(name="ep", bufs=3) as ep, \
             tc.tile_pool(name="ost", bufs=6) as ost, \
             tc.tile_pool(name="pbig", bufs=2, space="PSUM") as pbig, \
             tc.tile_pool(name="pctx", bufs=4, space="PSUM") as pctx:

            def emit_body():
                # ---- persistent SBUF tensors ----
                hsT_sb = big.tile([128, KC, S], BF16)
                wq_sb = big.tile([128, KC, DG], BF16)
                wk_sb = big.tile([128, KC, DG], BF16)
                wv_sb = big.tile([128, KC, DG], BF16)
                wo_sb = big.tile([128, 2, D], BF16)
                bq_sb = big.tile([128, 2], F32)
                bk_sb = big.tile([128, 2], F32)
                qT_sb = big.tile([128, 2, S], BF16)
                kT_sb = big.tile([128, 2, S], BF16)
                # DoubleRow-paired V: [sk(part), skc-pair, j, head, 80(64+ones)]
                v_sb = big.tile([128, SPC, 2, HG, 80], FP8)
                ctxT_sb = big.tile([128, 2, S], BF16)
                ones_f = big.tile([65, 64], F32)
                ones_r = big.tile([65, 64], F32R)
                vones_f = big.tile([128, SPC, 2, HG, 1], F32)

                # ---- input DMAs (ordered by first use) ----
                wk_r = wk[:, :].rearrange("(kc p) n -> p kc n", p=128)
                wq_r = wq[:, :].rearrange("(kc p) n -> p kc n", p=128)
                wv_r = wv[:, :].rearrange("(kc p) n -> p kc n", p=128)
                nc.sync.dma_start(out=bk_sb, in_=bk[:, :].rearrange("md p -> p md"))
                nc.sync.dma_start(out=bq_sb, in_=bq[:, :].rearrange("md p -> p md"))
                for kc in range(KC):
                    if kc % 2 == 0:
                        nc.sync.dma_start(out=wk_sb[:, kc:kc + 2, :], in_=wk_r[:, kc:kc + 2, :])
                    nc.sync.dma_start(
                        out=hsT_sb[:, kc, 0:512],
                        in_=hsT[kc * 128:(kc + 1) * 128, 0:512],
                    )
                for kc in range(KC):
                    if kc % 2 == 0:
                        nc.sync.dma_start(out=wq_sb[:, kc:kc + 2, :], in_=wq_r[:, kc:kc + 2, :])
                    nc.sync.dma_start(
                        out=hsT_sb[:, kc, 512:1024],
                        in_=hsT[kc * 128:(kc + 1) * 128, 512:1024],
                    )
                for kc in range(0, KC, 2):
                    nc.sync.dma_start(out=wv_sb[:, kc:kc + 2, :], in_=wv_r[:, kc:kc + 2, :])
                for sqc in range(2, SQC):
                    for kc in range(KC):
                        nc.sync.dma_start(
                            out=hsT_sb[:, kc, sqc * 512:(sqc + 1) * 512],
                            in_=hsT[kc * 128:(kc + 1) * 128, sqc * 512:(sqc + 1) * 512],
                        )
                wo_r = wo[:, :].rearrange("(kc p) n -> p kc n", p=128)
                for oc in range(0, D, 256):
                    nc.sync.dma_start(out=wo_sb[:, :, oc:oc + 256], in_=wo_r[:, :, oc:oc + 256])

                # ---- constants ----
                nc.vector.memset(ones_f, 1.0)
                nc.vector.tensor_copy(ones_r, ones_f)
                nc.vector.memset(vones_f, 1.0)
                nc.vector.tensor_copy(v_sb[:, :, :, :, 64:65], vones_f)

                # ---- helper emitters ----
                def qk_proj(w_sb, b_sb, dst, sqc, md):
                    ssl = slice(sqc * 512, (sqc + 1) * 512)
                    msl = slice(md * 128, (md + 1) * 128)
                    ps = pbig.tile([128, 512], F32, tag="st", name="ps_qk")
                    for kc in range(KC):
                        nc.tensor.matmul(
                            ps, w_sb[:, kc, msl], hsT_sb[:, kc, ssl],
                            start=(kc == 0), stop=(kc == KC - 1),
                        )
                    nc.scalar.activation(
                        dst[:, md, ssl], ps, AFT.Identity, bias=b_sb[:, md:md + 1],
                    )

                def v_proj(skc):
                    ksl = slice(skc * 128, (skc + 1) * 128)
                    psv = pbig.tile([128, DG], F32, tag="st", name="psv")
                    for kc in range(KC):
                        nc.tensor.matmul(
                            psv, hsT_sb[:, kc, ksl], wv_sb[:, kc, :],
                            start=(kc == 0), stop=(kc == KC - 1),
                        )
                    with nc.allow_low_precision(reason="v stored e4m3 for DoubleRow PV"):
                        nc.vector.tensor_copy(
                            v_sb[:, skc // 2, skc % 2, :, 0:64],
                            psv.rearrange("p (h d) -> p h d", h=HG),
                        )

                def epilogue(h, ctx, sqc):
                    # normalize ctxT_unnorm (rows 0:64) by l (row 64), write ctxT
                    poff = (h % 2) * 64
                    cpart = h // 2
                    ssl = slice(sqc * 512, (sqc + 1) * 512)
                    invl_r = ep.tile([65, 512], F32R, tag="invr", name="invl_r")
                    with nc.allow_low_precision(reason="1/l rounded to fp32r feeds the fp32r broadcast matmul"):
                        nc.vector.reciprocal(invl_r[64:65, :], ctx[64:65, :])
                    psb = pbig.tile([64, 512], F32, tag="st", name="psb")
                    nc.tensor.matmul(
                        psb, ones_r[64:65, 0:64], invl_r[64:65, :],
                        start=True, stop=True,
                    )
                    bc = ep.tile([64, 512], F32, tag="bc", name="bc")
                    nc.scalar.copy(out=bc, in_=psb)
                    with nc.allow_low_precision(reason="ctxT stored bf16 for the out-projection"):
                        nc.vector.tensor_mul(
                            ctxT_sb[poff:poff + 64, cpart, ssl],
                            ctx[0:64, :], bc,
                        )

                def out_proj(mo, sqc):
                    # evacuation on ScalarE: the DVE carries the slow
                    # InstReciprocal epilogues plus 7/16 of the exps, so
                    # ScalarE (9 exps + qk copies) takes the evacuations to
                    # equalize elementwise busy time.
                    osl = slice(mo * 128, (mo + 1) * 128)
                    ssl = slice(sqc * 512, (sqc + 1) * 512)
                    pso = pctx.tile([128, 512], F32, tag="ctx", name="pso")
                    for kc2 in range(2):
                        nc.tensor.matmul(
                            pso, wo_sb[:, kc2, osl], ctxT_sb[:, kc2, ssl],
                            start=(kc2 == 0), stop=(kc2 == 1),
                        )
                    ot = ost.tile([128, 512], BF16, name="ot")
                    with nc.allow_low_precision(reason="outT partials summed on host; bf16 halves the store DMA"):
                        nc.scalar.copy(out=ot, in_=pso)
                    nc.sync.dma_start(out=outT[osl, ssl], in_=ot)

                # ---- PE warmup (p-state ramp during initial DMA wait) ----
                warm = pbig.tile([128, 512], F32, tag="st", name="warm")
                for wi in range(36):
                    nc.tensor.matmul(
                        warm[0:64, 0:64], ones_r[0:64, 0:64], ones_r[0:64, 0:64],
                        start=(wi == 0), stop=(wi == 35),
                    )

                # ---- pre-attention projections: kT sqc 0-1, qT sqc 0 ----
                for sqc in range(2):
                    for md in range(2):
                        qk_proj(wk_sb, bk_sb, kT_sb, sqc, md)
                for md in range(2):
                    qk_proj(wq_sb, bq_sb, qT_sb, 0, md)

                from collections import deque
                prio_q = deque()
                slack_q = deque()

                def drip(slack_ok):
                    if prio_q:
                        prio_q.popleft()()
                    elif slack_ok and slack_q:
                        slack_q.popleft()()

                # ---- attention units: (sqc, head-pair), software-pipelined ----
                units = [(sqc, pair) for sqc in range(SQC) for pair in range(2)]
                for ui, (sqc, pair) in enumerate(units):
                    ssl = slice(sqc * 512, (sqc + 1) * 512)
                    ctx0 = pctx.tile([65, 512], F32, tag="ctx", name="ctx0")
                    ctx1 = pctx.tile([65, 512], F32, tag="ctx", name="ctx1")
                    ctxs = (ctx0, ctx1)
                    if ui == 0:
                        for k_sqc in (2, 3):
                            for md in range(2):
                                prio_q.append(
                                    lambda k_sqc=k_sqc, md=md: qk_proj(wk_sb, bk_sb, kT_sb, k_sqc, md))
                    # qT(s) is first needed by unit 2s; drip it one unit-pair
                    # ahead, spread across units 1/2/4 instead of all in unit 1
                    # so the PE load is flatter against the steady ScalarE/DVE
                    # exp stream.
                    if ui in (1, 2, 4):
                        q_sqc = {1: 1, 2: 2, 4: 3}[ui]
                        for md in range(2):
                            prio_q.append(
                                lambda q_sqc=q_sqc, md=md: qk_proj(wq_sb, bq_sb, qT_sb, q_sqc, md))
                    expT = None
                    prev_pair = None
                    for skc in range(SKC):
                        if ui == 0:
                            v_proj(skc)  # stream the v projection under unit 0
                        ksl = slice(skc * 128, (skc + 1) * 128)
                        sT = pbig.tile([128, 1024], F32, tag="st", name="sT")
                        for hh in range(2):
                            nc.tensor.matmul(
                                sT[:, hh * 512:(hh + 1) * 512],
                                kT_sb[hh * 64:(hh + 1) * 64, pair, ksl],
                                qT_sb[hh * 64:(hh + 1) * 64, pair, ssl],
                                start=True, stop=True,
                            )
                        if skc % 2 == 0:
                            expT = expp.tile([128, 2, 2, 512], FP8, name="expT")
                        eslot = expT[:, skc % 2, :, :]
                        if skc in DVE_EXP_SKC:
                            with nc.allow_low_precision(reason="Schraudolph exp to e4m3 on DVE"):
                                nc.vector.tensor_scalar(
                                    out=eslot.bitcast(I8), in0=sT,
                                    scalar1=float(SCH_C1), scalar2=float(SCH_C2),
                                    op0=ALU.mult, op1=ALU.add,
                                )
                        else:
                            with nc.allow_low_precision(reason="exp stored e4m3 for DoubleRow PV"):
                                nc.scalar.activation(eslot, sT, AFT.Exp, scale=SCALE)
                        if skc % 2 == 1:
                            # PV for this completed chunk pair (DoubleRow fp8)
                            sp = skc // 2
                            for hh in range(2):
                                nc.tensor.matmul(
                                    ctxs[hh], v_sb[:, sp, :, pair * 2 + hh, 0:65],
                                    expT[:, :, hh, :],
                                    start=(sp == 0), stop=(sp == SPC - 1),
                                    perf_mode=DR,
                                )
                        if skc >= 2:
                            drip(slack_ok=(skc % 2 == 0))
                        prev_pair = skc // 2
                    del prev_pair
                    # defer this unit's epilogues into the next unit's loop
                    for hh in range(2):
                        prio_q.append(
                            lambda h=pair * 2 + hh, ctx=ctxs[hh], sqc=sqc: epilogue(h, ctx, sqc))
                    if pair == 1:
                        for mo in range(8):
                            slack_q.append(lambda mo=mo, sqc=sqc: out_proj(mo, sqc))

                # drain remaining deferred work
                while prio_q or slack_q:
                    drip(slack_ok=True)

            for _rep in range(reps):
                emit_body()

    nc.compile()
    return nc


def _get_nc(reps=1):
    global _cached_nc
    if reps != 1:
        return _build_nc(reps)
    if _cached_nc is None:
        _cached_nc = _build_nc()
    return _cached_nc


def kernel(hidden_states, Wq, bq, Wk, bk, Wv, bv, Wo, bo, _want_trace=False):
    import ml_dtypes
    from concourse.bass_utils import run_bass_kernel_spmd

    bf16 = ml_dtypes.bfloat16
    hidden_states = np.asarray(hidden_states, dtype=np.float32)
    Wq = np.asarray(Wq, dtype=np.float32)
    Wk = np.asarray(Wk, dtype=np.float32)
    Wv = np.asarray(Wv, dtype=np.float32)
    Wo = np.asarray(Wo, dtype=np.float32)
    bq = np.asarray(bq, dtype=np.float32)
    bk = np.asarray(bk, dtype=np.float32)
    bv = np.asarray(bv, dtype=np.float32)
    bo = np.asarray(bo, dtype=np.float32)

    nc = _get_nc()

    hsTs = [np.ascontiguousarray(hidden_states[b].T).astype(bf16) for b in range(B)]
    in_maps = []
    for c in range(N_CORES):
        b, hg = divmod(c, HG)
        sl = slice(hg * DG, (hg + 1) * DG)
        in_maps.append({
            "hsT": hsTs[b],
            "wq": np.ascontiguousarray(Wq[:, sl]).astype(bf16),
            "wk": np.ascontiguousarray(Wk[:, sl]).astype(bf16),
            "wv": np.ascontiguousarray(Wv[:, sl]).astype(bf16),
            "wo": np.ascontiguousarray(Wo[sl, :]).astype(bf16),
            "bq": np.ascontiguousarray(bq[sl].reshape(2, 128)),
            "bk": np.ascontiguousarray(bk[sl].reshape(2, 128)),
        })

    try:
        res = run_bass_kernel_spmd(
            nc, in_maps, core_ids=list(range(N_CORES)), trace=_want_trace,
        )
    except ModuleNotFoundError:
        res = run_bass_kernel_spmd(
            nc, in_maps, core_ids=list(range(N_CORES)), trace=False,
        )

    bias_full = bo + bv @ Wo  # [D]
    out = np.empty((B, S, D), dtype=np.float32)
    for b in range(B):
        acc = res.results[HG * b]["outT"].astype(np.float32)
        for g in range(1, HG):
            acc = acc + res.results[HG * b + g]["outT"].astype(np.float32)
        out[b] = acc.T + bias_full

    if _want_trace:
        return out, res
    return out



# revision 11
# speedup vs baseline: 1.1157x; 1.1157x over previous
"""Multi-head attention (B=2, S=2048, H=16, DH=64, D=1024) on 8 TRN2 NeuronCores.

Sharding: batch x head-group. Core c handles batch b = c//4, head group
hg = c%4 (4 heads = 256 hidden columns). Each core computes its head group's
attention and a partial (row-sliced) output projection; the host sums the 4
partials per batch and adds the bias terms.

Device-side dataflow (per core), v2:
  - All projection operands (hsT, Wq/Wk/Wv/Wo) arrive bf16 (halves input DMA
    vs fp32r; PE rate identical). PSUM accumulation fp32 throughout.
  - qT/kT [256, S] bf16 via Wq/Wk-contract matmuls; bias added by the ScalarE
    Identity activation that evacuates PSUM.
  - v = hs @ Wv stored as fp8(e4m3) in a DoubleRow-paired layout
    v_sb[128, sp, j, h, 80]: sk-chunk pair sp = skc//2, j = skc%2, with a
    ones column at col 64 per (j, h) slot so the PV matmul also produces the
    softmax denominator l as ctx row 64.
  - Per head pair and sk chunk: two K=64 scoresT matmuls (bf16) -> one
    [128, 1024] psum tile; exp is computed chunk-by-chunk into a paired fp8
    tile expT[128, j, head, 512]: most chunks on ScalarE (Exp activation,
    fp8 out), a subset on the DVE as a one-instruction Schraudolph
    exp-to-e4m3: int8(x*(SCALE*8/ln2) + 56.x) bitcast to fp8 (sawtooth err
    ~2-3%, cancelled to first order by the shared denominator).
  - PV: fp8 DoubleRow matmuls, K_eff=256 (chunk pair packed along the free
    dim of both operands): lhsT = v_sb[:, sp, :, h, 0:65], rhs =
    expT[:, :, h, :] -> ctx [65, 512] accumulated over 8 pairs.
  - Epilogue per head: l -> 1/l (DVE reciprocal), broadcast across 64
    partitions via a K=1 matmul, ctxT(bf16) = ctx_unnorm * bcast(1/l).
  - outT_partial [D, S] fp32 = Wo-contract over ctxT (bf16).
Software-pipelined identically to v1 (deferred qT/kT projections, epilogues
and out-projections dripped into later attention units' skc loops).
Host: out[b] = sum_hg(outT_partial).T + (bo + bv @ Wo).
"""

import numpy as np

H = 16
DH = 64
D = 1024
B = 2
S = 2048
HG = 4            # heads per core
DG = HG * DH      # 256 hidden cols per core
SCALE = DH ** -0.5
N_CORES = 8

# Each skc's score tile is split per head-half; the half with
# (skc + hh) % 2 == 1 runs Schraudolph exp on the DVE, the other half exact
# exp on ScalarE — both concurrently, halving the sT free latency.
# Schraudolph-to-e4m3 constants: code = x*SCALE*(2^3/ln2) + (7*2^3 - c)
SCH_C1 = SCALE * 8.0 / float(np.log(2.0))
SCH_C2 = 56.0 - 0.40  # sawtooth centering; argmin of the full-pipeline numpy
                      # model (parity=1 checkerboard: rel 0.0171)

_cached_nc = None


def _build_nc(reps=1):
    import concourse.bass as bass  # noqa: F401
    from concourse import bacc
    import concourse.mybir as mybir
    import concourse.tile as tile

    F32 = mybir.dt.float32
    F32R = mybir.dt.float32r
    BF16 = mybir.dt.bfloat16
    FP8 = mybir.dt.float8e4
    I8 = mybir.dt.int8
    AFT = mybir.ActivationFunctionType
    ALU = mybir.AluOpType
    DR = mybir.MatmulPerfMode.DoubleRow

    nc = bacc.Bacc("TRN2", target_bir_lowering=False)

    hsT = nc.dram_tensor("hsT", [D, S], BF16, kind="ExternalInput")
    wq = nc.dram_tensor("wq", [D, DG], BF16, kind="ExternalInput")
    wk = nc.dram_tensor("wk", [D, DG], BF16, kind="ExternalInput")
    wv = nc.dram_tensor("wv", [D, DG], BF16, kind="ExternalInput")
    wo = nc.dram_tensor("wo", [DG, D], BF16, kind="ExternalInput")
    bq = nc.dram_tensor("bq", [2, 128], F32, kind="ExternalInput")
    bk = nc.dram_tensor("bk", [2, 128], F32, kind="ExternalInput")
    outT = nc.dram_tensor("outT", [D, S], BF16, kind="ExternalOutput")

    KC = D // 128     # 8 contraction chunks for projections
    SQC = S // 512    # 4 sq chunks of 512
    SKC = S // 128    # 16 sk chunks of 128
    SPC = SKC // 2    # 8 sk chunk-pairs (DoubleRow)

    with tile.TileContext(nc) as tc:
        with tc.tile_pool(name="big", bufs=1) as big, \
             tc.tile_pool(name="expp", bufs=4) as expp, \
             tc.tile_pool(name="ep", bufs=3) as ep, \
             tc.tile_pool(name="ost", bufs=6) as ost, \
             tc.tile_pool(name="pbig", bufs=4, space="PSUM") as pbig, \
             tc.tile_pool(name="pctx", bufs=4, space="PSUM") as pctx:

            def emit_body():
                # ---- persistent SBUF tensors ----
                hsT_sb = big.tile([128, KC, S], BF16)
                wq_sb = big.tile([128, KC, DG], BF16)
                wk_sb = big.tile([128, KC, DG], BF16)
                wv_sb = big.tile([128, KC, DG], BF16)
                wo_sb = big.tile([128, 2, D], BF16)
                bq_sb = big.tile([128, 2], F32)
                bk_sb = big.tile([128, 2], F32)
                qT_sb = big.tile([128, 2, S], BF16)
                kT_sb = big.tile([128, 2, S], BF16)
                # DoubleRow-paired V: [sk(part), skc-pair, j, head, 80(64+ones)]
                v_sb = big.tile([128, SPC, 2, HG, 80], FP8)
                ctxT_sb = big.tile([128, 2, S], BF16)
                ones_f = big.tile([65, 64], F32)
                ones_r = big.tile([65, 64], F32R)
                vones_f = big.tile([128, SPC, 2, HG, 1], F32)

                # ---- input DMAs (ordered by first use) ----
                wk_r = wk[:, :].rearrange("(kc p) n -> p kc n", p=128)
                wq_r = wq[:, :].rearrange("(kc p) n -> p kc n", p=128)
                wv_r = wv[:, :].rearrange("(kc p) n -> p kc n", p=128)
                nc.sync.dma_start(out=bk_sb, in_=bk[:, :].rearrange("md p -> p md"))
                nc.sync.dma_start(out=bq_sb, in_=bq[:, :].rearrange("md p -> p md"))
                for kc in range(KC):
                    if kc % 2 == 0:
                        nc.sync.dma_start(out=wk_sb[:, kc:kc + 2, :], in_=wk_r[:, kc:kc + 2, :])
                    nc.sync.dma_start(
                        out=hsT_sb[:, kc, 0:512],
                        in_=hsT[kc * 128:(kc + 1) * 128, 0:512],
                    )
                for kc in range(KC):
                    if kc % 2 == 0:
                        nc.sync.dma_start(out=wq_sb[:, kc:kc + 2, :], in_=wq_r[:, kc:kc + 2, :])
                    nc.sync.dma_start(
                        out=hsT_sb[:, kc, 512:1024],
                        in_=hsT[kc * 128:(kc + 1) * 128, 512:1024],
                    )
                for kc in range(0, KC, 2):
                    nc.sync.dma_start(out=wv_sb[:, kc:kc + 2, :], in_=wv_r[:, kc:kc + 2, :])
                for sqc in range(2, SQC):
                    for kc in range(KC):
                        nc.sync.dma_start(
                            out=hsT_sb[:, kc, sqc * 512:(sqc + 1) * 512],
                            in_=hsT[kc * 128:(kc + 1) * 128, sqc * 512:(sqc + 1) * 512],
                        )
                wo_r = wo[:, :].rearrange("(kc p) n -> p kc n", p=128)
                for oc in range(0, D, 256):
                    nc.sync.dma_start(out=wo_sb[:, :, oc:oc + 256], in_=wo_r[:, :, oc:oc + 256])

                # ---- constants ----
                nc.vector.memset(ones_f, 1.0)
                nc.vector.tensor_copy(ones_r, ones_f)
                nc.vector.memset(vones_f, 1.0)
                nc.vector.tensor_copy(v_sb[:, :, :, :, 64:65], vones_f)

                # ---- helper emitters ----
                def qk_proj_parts(w_sb, b_sb, dst, sqc, md):
                    # split into two drip payloads (~0.9us PE each) so a drip
                    # never monopolizes the PE long enough to stall the score
                    # chain; the ps tile comes from the pctx rotation so the
                    # sT pool stays a dedicated depth-4 score-chain rotation
                    ssl = slice(sqc * 512, (sqc + 1) * 512)
                    msl = slice(md * 128, (md + 1) * 128)
                    state = {}

                    def part1():
                        ps = pctx.tile([128, 512], F32, tag="ctx", name="ps_qk")
                        state["ps"] = ps
                        for kc in range(KC // 2):
                            nc.tensor.matmul(
                                ps, w_sb[:, kc, msl], hsT_sb[:, kc, ssl],
                                start=(kc == 0), stop=False,
                            )

                    def part2():
                        ps = state["ps"]
                        for kc in range(KC // 2, KC):
                            nc.tensor.matmul(
                                ps, w_sb[:, kc, msl], hsT_sb[:, kc, ssl],
                                start=False, stop=(kc == KC - 1),
                            )
                        nc.scalar.activation(
                            dst[:, md, ssl], ps, AFT.Identity, bias=b_sb[:, md:md + 1],
                        )

                    return [part1, part2]

                def qk_proj(w_sb, b_sb, dst, sqc, md):
                    for part in qk_proj_parts(w_sb, b_sb, dst, sqc, md):
                        part()

                def v_proj(skc):
                    ksl = slice(skc * 128, (skc + 1) * 128)
                    psv = pctx.tile([128, DG], F32, tag="ctx", name="psv")
                    for kc in range(KC):
                        nc.tensor.matmul(
                            psv, hsT_sb[:, kc, ksl], wv_sb[:, kc, :],
                            start=(kc == 0), stop=(kc == KC - 1),
                        )
                    with nc.allow_low_precision(reason="v stored e4m3 for DoubleRow PV"):
                        nc.vector.tensor_copy(
                            v_sb[:, skc // 2, skc % 2, :, 0:64],
                            psv.rearrange("p (h d) -> p h d", h=HG),
                        )

                def epilogue(h, ctx, sqc):
                    # normalize ctxT_unnorm (rows 0:64) by l (row 64), write ctxT
                    poff = (h % 2) * 64
                    cpart = h // 2
                    ssl = slice(sqc * 512, (sqc + 1) * 512)
                    invl = ep.tile([1, 512], F32, tag="invr", name="invl")
                    nc.vector.reciprocal(invl, ctx[64:65, :])
                    # broadcast 1/l across 64 partitions on the (otherwise
                    # idle) Pool engine: replaces a K=1 PE matmul + ACT copy
                    bc = ep.tile([64, 512], F32, tag="bc", name="bc")
                    nc.gpsimd.partition_broadcast(bc, invl)
                    with nc.allow_low_precision(reason="ctxT stored bf16 for the out-projection"):
                        nc.vector.tensor_mul(
                            ctxT_sb[poff:poff + 64, cpart, ssl],
                            ctx[0:64, :], bc,
                        )

                def out_proj(mo, sqc):
                    # evacuation on ScalarE: the DVE carries the slow
                    # InstReciprocal epilogues plus 7/16 of the exps, so
                    # ScalarE (9 exps + qk copies) takes the evacuations to
                    # equalize elementwise busy time.
                    osl = slice(mo * 128, (mo + 1) * 128)
                    ssl = slice(sqc * 512, (sqc + 1) * 512)
                    pso = pctx.tile([128, 512], F32, tag="ctx", name="pso")
                    for kc2 in range(2):
                        nc.tensor.matmul(
                            pso, wo_sb[:, kc2, osl], ctxT_sb[:, kc2, ssl],
                            start=(kc2 == 0), stop=(kc2 == 1),
                        )
                    ot = ost.tile([128, 512], BF16, name="ot")
                    with nc.allow_low_precision(reason="outT partials summed on host; bf16 halves the store DMA"):
                        nc.scalar.copy(out=ot, in_=pso)
                    nc.sync.dma_start(out=outT[osl, ssl], in_=ot)

                # ---- PE warmup (p-state ramp during initial DMA wait) ----
                warm = pctx.tile([128, 512], F32, tag="ctx", name="warm")
                for wi in range(36):
                    nc.tensor.matmul(
                        warm[0:64, 0:64], ones_r[0:64, 0:64], ones_r[0:64, 0:64],
                        start=(wi == 0), stop=(wi == 35),
                    )

                # ---- pre-attention projections: kT sqc 0-1, qT sqc 0 ----
                for sqc in range(2):
                    for md in range(2):
                        qk_proj(wk_sb, bk_sb, kT_sb, sqc, md)
                for md in range(2):
                    qk_proj(wq_sb, bq_sb, qT_sb, 0, md)

                from collections import deque
                prio_q = deque()
                slack_q = deque()

                def drip(slack_ok):
                    if prio_q:
                        prio_q.popleft()()
                    elif slack_ok and slack_q:
                        slack_q.popleft()()

                # ---- attention units: (sqc, head-pair), software-pipelined ----
                units = [(sqc, pair) for sqc in range(SQC) for pair in range(2)]
                for ui, (sqc, pair) in enumerate(units):
                    ssl = slice(sqc * 512, (sqc + 1) * 512)
                    ctx0 = pctx.tile([65, 512], F32, tag="ctx", name="ctx0")
                    ctx1 = pctx.tile([65, 512], F32, tag="ctx", name="ctx1")
                    ctxs = (ctx0, ctx1)
                    if ui == 0:
                        for k_sqc in (2, 3):
                            for md in range(2):
                                prio_q.extend(
                                    qk_proj_parts(wk_sb, bk_sb, kT_sb, k_sqc, md))
                    # qT(s) is first needed by unit 2s; drip it one unit-pair
                    # ahead, spread across units 1/2/4 instead of all in unit 1
                    # so the PE load is flatter against the steady ScalarE/DVE
                    # exp stream.
                    if ui in (1, 2, 4):
                        q_sqc = {1: 1, 2: 2, 4: 3}[ui]
                        for md in range(2):
                            prio_q.extend(
                                qk_proj_parts(wq_sb, bq_sb, qT_sb, q_sqc, md))
                    expT = None
                    for skc in range(SKC):
                        if ui == 0:
                            v_proj(skc)  # stream the v projection under unit 0
                        ksl = slice(skc * 128, (skc + 1) * 128)
                        if skc % 2 == 0:
                            expT = expp.tile([128, 2, 2, 512], FP8, name="expT")
                        # per-head score tiles: 4-deep rotation, and each
                        # half's exp runs concurrently on DVE/ScalarE so the
                        # tile frees after ~0.6us instead of ~1.1us
                        for hh in range(2):
                            sT = pbig.tile([128, 512], F32, tag="st", name="sT")
                            nc.tensor.matmul(
                                sT,
                                kT_sb[hh * 64:(hh + 1) * 64, pair, ksl],
                                qT_sb[hh * 64:(hh + 1) * 64, pair, ssl],
                                start=True, stop=True,
                            )
                            eslot = expT[:, skc % 2, hh, :]
                            if (skc + hh) % 2 == 1:
                                with nc.allow_low_precision(reason="Schraudolph exp to e4m3 on DVE"):
                                    nc.vector.tensor_scalar(
                                        out=eslot.bitcast(I8), in0=sT,
                                        scalar1=float(SCH_C1), scalar2=float(SCH_C2),
                                        op0=ALU.mult, op1=ALU.add,
                                    )
                            else:
                                with nc.allow_low_precision(reason="exp stored e4m3 for DoubleRow PV"):
                                    nc.scalar.activation(eslot, sT, AFT.Exp, scale=SCALE)
                        if skc % 2 == 1:
                            # PV for this completed chunk pair (DoubleRow fp8)
                            sp = skc // 2
                            for hh in range(2):
                                nc.tensor.matmul(
                                    ctxs[hh], v_sb[:, sp, :, pair * 2 + hh, 0:65],
                                    expT[:, :, hh, :],
                                    start=(sp == 0), stop=(sp == SPC - 1),
                                    perf_mode=DR,
                                )
                        if skc >= 2:
                            drip(slack_ok=(skc % 2 == 0))
                    # defer this unit's epilogues into the next unit's loop
                    for hh in range(2):
                        prio_q.append(
                            lambda h=pair * 2 + hh, ctx=ctxs[hh], sqc=sqc: epilogue(h, ctx, sqc))
                    if pair == 1:
                        for mo in range(8):
                            slack_q.append(lambda mo=mo, sqc=sqc: out_proj(mo, sqc))

                # drain remaining deferred work
                while prio_q or slack_q:
                    drip(slack_ok=True)

            for _rep in range(reps):
                emit_body()

    nc.compile()
    return nc


def _get_nc(reps=1):
    global _cached_nc
    if reps != 1:
        return _build_nc(reps)
    if _cached_nc is None:
        _cached_nc = _build_nc()
    return _cached_nc


def kernel(hidden_states, Wq, bq, Wk, bk, Wv, bv, Wo, bo, _want_trace=False):
    import ml_dtypes
    from concourse.bass_utils import run_bass_kernel_spmd

    bf16 = ml_dtypes.bfloat16
    hidden_states = np.asarray(hidden_states, dtype=np.float32)
    Wq = np.asarray(Wq, dtype=np.float32)
    Wk = np.asarray(Wk, dtype=np.float32)
    Wv = np.asarray(Wv, dtype=np.float32)
    Wo = np.asarray(Wo, dtype=np.float32)
    bq = np.asarray(bq, dtype=np.float32)
    bk = np.asarray(bk, dtype=np.float32)
    bv = np.asarray(bv, dtype=np.float32)
    bo = np.asarray(bo, dtype=np.float32)

    nc = _get_nc()

    hsTs = [np.ascontiguousarray(hidden_states[b].T).astype(bf16) for b in range(B)]
    in_maps = []
    for c in range(N_CORES):
        b, hg = divmod(c, HG)
        sl = slice(hg * DG, (hg + 1) * DG)
        in_maps.append({
            "hsT": hsTs[b],
            "wq": np.ascontiguousarray(Wq[:, sl]).astype(bf16),
            "wk": np.ascontiguousarray(Wk[:, sl]).astype(bf16),
            "wv": np.ascontiguousarray(Wv[:, sl]).astype(bf16),
            "wo": np.ascontiguousarray(Wo[sl, :]).astype(bf16),
            "bq": np.ascontiguousarray(bq[sl].reshape(2, 128)),
            "bk": np.ascontiguousarray(bk[sl].reshape(2, 128)),
        })

    try:
        res = run_bass_kernel_spmd(
            nc, in_maps, core_ids=list(range(N_CORES)), trace=_want_trace,
        )
    except ModuleNotFoundError:
        res = run_bass_kernel_spmd(
            nc, in_maps, core_ids=list(range(N_CORES)), trace=False,
        )

    bias_full = bo + bv @ Wo  # [D]
    out = np.empty((B, S, D), dtype=np.float32)
    for b in range(B):
        acc = res.results[HG * b]["outT"].astype(np.float32)
        for g in range(1, HG):
            acc = acc + res.results[HG * b + g]["outT"].astype(np.float32)
        out[b] = acc.T + bias_full

    if _want_trace:
        return out, res
    return out

